# revision 62
# baseline (speedup 1.0000x reference)
"""Multi-Head Latent Attention (GQA, causal) on 8 Trainium2 NeuronCores.

Sharding: tensor-parallel by heads. Core c owns query heads 4c..4c+3 and
kv head c. Big projections (q, kc/vc down, o) run as fp8-e4m3 DoubleRow
matmuls (2 k-tiles per instruction at 0.5 cyc/row = 4x bf16) in a 3-term
residual-split form  a@W ~= a8@W8 + ra8@W8 + a8@RW8  which restores
~13-bit mantissa accuracy at 0.75x the bf16 cycle cost. All fp8 tensors
are pre-scaled by power-of-2 factors into e4m3's normal range; the
inverse scales fold into the psum->sbuf copies. Attention itself
(scores, exp, AV) and the small up-projections stay bf16:
  - scores^T[t,q] come straight out of the PE in the layout the AV
    matmul consumes (no per-block P transposes),
  - the AV matmul's 129th column (a constant 1/32 column of V) yields
    the softmax denominator pre-multiplied so the per-partition
    normalization also applies the fp8 scale of the attention output,
  - one 128x128 PE transpose per (head, q-tile) restores [d,q], from
    which both the fp8 attention-out tile and its fp8 residual are
    derived for the o-projection,
  - the partial o-projection (input dims c*512..(c+1)*512) is
    interleaved into the NEXT chunk's attention loop so the PE has
    filler work during exp-bound stretches.
The host sums the 8 partial outputs (the all-reduce after o_proj).
"""

import sys

import ml_dtypes
import numpy as np

if "/opt/trn_rl_repo" not in sys.path:
    sys.path.insert(0, "/opt/trn_rl_repo")

B, S, HID = 1, 2048, 4096
H, HK, D = 32, 8, 128
L = D // 4  # 32
NCORE = 8
HPC = H // NCORE  # 4 query heads per core
NKT = HID // 128  # 32 k-tiles over hidden dim
CHUNK = 512
NCHUNK = S // CHUNK  # 4
NSQ = S // 128  # 16 sq tiles
SSH = S // NCORE  # 256

# power-of-2 fp8 range scales
A_HS = 16.0      # hidden states
A_WQ = 8192.0    # Wq * (1/sqrt D)
A_WD = 1024.0    # Wk_down / Wv_down
A_WO = 1024.0    # Wo
A_AT = 32.0      # attention output (av/den)
Q_COPY = 1.0 / (A_HS * A_WQ)   # q psum -> bf16 qT
CV_COPY = 1.0 / (A_HS * A_WD)  # kc/vc psum -> bf16
O_COPY = 1.0 / (A_AT * A_WO)   # o psum -> bf16 out

_BUILT = None


def _build():
    import concourse.mybir as mybir
    import concourse.tile as tile
    from concourse import bacc

    f32 = mybir.dt.float32
    bf16 = mybir.dt.bfloat16
    fp8 = mybir.dt.float8e4
    EXP = mybir.ActivationFunctionType.Exp
    DR = mybir.MatmulPerfMode.DoubleRow

    nc = bacc.Bacc()

    # value+residual pairs interleaved in one tensor: one DMA feeds both
    htc = nc.dram_tensor("htc", [HID // 128, 128, 2, S], fp8, kind="ExternalInput")
    hmc = nc.dram_tensor("hmc", [128, NKT, 2, SSH], fp8, kind="ExternalInput")
    wqc = nc.dram_tensor("wqc", [128, NKT, 2, HPC * D], fp8, kind="ExternalInput")
    # kc/vc down weights: [p, k, (wkd|rwkd|wvd|rwvd), 256]
    wdc = nc.dram_tensor("wdc", [128, NKT, 4, HK * L], fp8, kind="ExternalInput")
    wkup = nc.dram_tensor("wkup", [128, 8 * D], bf16, kind="ExternalInput")
    wvup = nc.dram_tensor("wvup", [128, 8 * D], bf16, kind="ExternalInput")
    woc = nc.dram_tensor("woc", [128, HPC, 2, HID], fp8, kind="ExternalInput")
    maskt = nc.dram_tensor("maskt", [128, 128], bf16, kind="ExternalInput")
    ident = nc.dram_tensor("ident", [128, 128], bf16, kind="ExternalInput")
    outp = nc.dram_tensor("out", [S, HID], bf16, kind="ExternalOutput")
    # kc/vc shard exchange: [p, tgt*512 + m*256 + u] per core -> gathered
    cv_bounce = nc.dram_tensor("cv_bounce", [128, 1024], bf16)
    cv_gath = nc.dram_tensor("cv_gath", [NCORE, 128, 1024], bf16, addr_space="Shared")

    with tile.TileContext(nc) as tc:
        with (
            tc.tile_pool(name="weights", bufs=1) as wpool,
            tc.tile_pool(name="persist", bufs=1) as ppool,
            tc.tile_pool(name="stream", bufs=10) as spool,
            tc.tile_pool(name="outs", bufs=6) as opool,
        ):
            # ---- constants + resident weights (most loads are deferred
            #      behind the cv bounce to keep the early DMA window clear) ----
            mask_sb = wpool.tile([128, 128], bf16)
            id_sb = wpool.tile([128, 128], bf16)
            wkup_sb = wpool.tile([128, 8 * D], bf16)
            wvup_sb = wpool.tile([128, 8 * D], bf16)
            wo_sb = wpool.tile([128, HPC, 2, HID], fp8)

            # ---- persistent activations ----
            qT = ppool.tile([128, HPC, S], bf16)  # [d, head, s] (q * 1/sqrt D)
            kcT = ppool.tile([128, 2, S], bf16)  # [latent%128, latent//128, s]
            vcT = ppool.tile([128, 2, S], bf16)
            kT = ppool.tile([128, S], bf16)  # [d, t] for our kv head
            # [t%128, t//128, d]; col 128 is a constant 1/A_AT column so the
            # AV matmul also produces the softmax denominator pre-divided by
            # A_AT (the attention-out fp8 scale)
            v_sb = ppool.tile([128, NSQ, 132], bf16)

            ht_r = htc.rearrange("k p two s -> p k two s")

            def q_matmuls(ps_q, wq_sb, h8, hr8, kp):
                """3-term fp8 DR matmuls for one k-pair of a q chunk."""
                st = dict(start=(kp == 0), stop=False)
                en = dict(start=False, stop=(kp == NKT // 2 - 1))
                ks = slice(2 * kp, 2 * kp + 2)
                for m in range(HPC):
                    ms = slice(m * 128, (m + 1) * 128)
                    nc.tensor.matmul(
                        ps_q[m][:], lhsT=wq_sb[:, ks, 0, ms], rhs=h8, perf_mode=DR,
                        **st,
                    )
                    nc.tensor.matmul(
                        ps_q[m][:], lhsT=wq_sb[:, ks, 0, ms], rhs=hr8, perf_mode=DR,
                        start=False, stop=False,
                    )
                    nc.tensor.matmul(
                        ps_q[m][:], lhsT=wq_sb[:, ks, 1, ms], rhs=h8, perf_mode=DR,
                        **en,
                    )

            # ---- phase A: cv seq-shard FIRST (PE ramps on it), so the
            #      bounce + 67us AllGather hide completely under the q
            #      projection that follows. DMA priority: cv inputs, then
            #      wq + constants + ht slabs on the scalar queue ----
            with (
                tc.tile_pool(name="qw", bufs=1) as qwpool,
                tc.tile_pool(name="psq", bufs=1, space="PSUM") as psq,
            ):
                wq_sb = qwpool.tile([128, NKT, 2, HPC * D], fp8)
                with (
                    tc.tile_pool(name="b0", bufs=1) as bpool,
                    tc.tile_pool(name="psb0", bufs=1, space="PSUM") as psb0,
                ):
                    wd_sb = bpool.tile([128, NKT, 4, HK * L], fp8)
                    hm_sb = bpool.tile([128, NKT, 2, SSH], fp8)
                    # 8 groups of the k range so cv matmuls start early.
                    # NOTHING else is issued before the bounce: the DMA
                    # device is a strict-FIFO, so any q-side load issued now
                    # would delay the bounce and with it the 67us AllGather.
                    cvg = [0, 2, 8, 20, 32]
                    for lo, hi in zip(cvg, cvg[1:]):
                        ks = slice(lo, hi)
                        nc.sync.dma_start(out=hm_sb[:, ks], in_=hmc[:, ks])
                        nc.sync.dma_start(out=wd_sb[:, ks], in_=wdc[:, ks])
                    # cv seq-shard (3-term fp8 DR for both kc and vc)
                    ps_cv = [
                        psb0.tile([128, SSH], f32, tag=f"ps_cv{t}", name=f"ps_cv{t}")
                        for t in range(4)
                    ]
                    for kp in range(NKT // 2):
                        ks = slice(2 * kp, 2 * kp + 2)
                        st = dict(start=(kp == 0), stop=False)
                        en = dict(start=False, stop=(kp == NKT // 2 - 1))
                        for ti in range(2):  # kc, vc
                            for m in range(2):
                                ms = slice(m * 128, (m + 1) * 128)
                                ps = ps_cv[ti * 2 + m]
                                nc.tensor.matmul(
                                    ps[:], lhsT=wd_sb[:, ks, 2 * ti, ms],
                                    rhs=hm_sb[:, ks, 0, :], perf_mode=DR, **st,
                                )
                                nc.tensor.matmul(
                                    ps[:], lhsT=wd_sb[:, ks, 2 * ti, ms],
                                    rhs=hm_sb[:, ks, 1, :], perf_mode=DR,
                                    start=False, stop=False,
                                )
                                nc.tensor.matmul(
                                    ps[:], lhsT=wd_sb[:, ks, 2 * ti + 1, ms],
                                    rhs=hm_sb[:, ks, 0, :], perf_mode=DR, **en,
                                )
                    cvst = bpool.tile([128, 1024], bf16)
                    for t in range(4):
                        ti, m = t // 2, t % 2
                        dst = cvst[:, ti * 512 + m * 256 : ti * 512 + (m + 1) * 256]
                        nc.vector.tensor_scalar_mul(dst, ps_cv[t][:], CV_COPY)
                    nc.sync.dma_start(out=cv_bounce[:], in_=cvst[:])
                    # q-side loads start only now (behind the bounce in the
                    # DMA FIFO)
                    qg = [0, 4, 8, 16, 32]
                    for lo, hi in zip(qg, qg[1:]):
                        nc.sync.dma_start(out=wq_sb[:, lo:hi], in_=wqc[:, lo:hi])
                    nc.scalar.dma_start(out=mask_sb[:], in_=maskt[:])
                    nc.scalar.dma_start(out=id_sb[:], in_=ident[:])
                    nc.scalar.dma_start(out=wkup_sb[:], in_=wkup[:])
                    nc.scalar.dma_start(out=wvup_sb[:], in_=wvup[:])

                nc.gpsimd.collective_compute(
                    "AllGather",
                    mybir.AluOpType.bypass,
                    replica_groups=[list(range(NCORE))],
                    ins=[cv_bounce[:]],
                    outs=[cv_gath[:]],
                )
                # kc/vc distribution + wo ride the sync queue behind the gather
                g_r = cv_gath.rearrange("r p (t m u) -> t p m r u", t=2, m=2)
                for m in range(2):
                    nc.sync.dma_start(
                        out=kcT[:, m, :].rearrange("p (r u) -> p r u", r=NCORE),
                        in_=g_r[0, :, m],
                    )
                    nc.sync.dma_start(
                        out=vcT[:, m, :].rearrange("p (r u) -> p r u", r=NCORE),
                        in_=g_r[1, :, m],
                    )
                for g in range(HPC):
                    nc.sync.dma_start(out=wo_sb[:, g], in_=woc[:, g])

                # ---- phase B: q chunks 0-3; combined ht slabs (4 k-pairs,
                #      value+residual) alternate Pool / scalar queues ----
                with tc.tile_pool(name="psq2", bufs=1, space="PSUM") as psq2:
                    for sc in range(NCHUNK):
                        pool = psq2 if sc % 2 == 0 else psq
                        ps_q = [
                            pool.tile(
                                [128, CHUNK], f32, tag=f"ps_q{m}", name=f"ps_q{m}"
                            )
                            for m in range(HPC)
                        ]
                        cs = slice(sc * CHUNK, (sc + 1) * CHUNK)
                        for sl in range(4):  # 4 slabs of 4 k-pairs
                            hch = spool.tile(
                                [128, 8, 2, CHUNK], fp8, tag="hch", name="hch",
                                bufs=4,
                            )
                            kslab = slice(8 * sl, 8 * sl + 8)
                            eng = nc.gpsimd if (4 * sc + sl) % 2 == 0 else nc.scalar
                            eng.dma_start(out=hch[:], in_=ht_r[:, kslab, :, cs])
                            for kk in range(4):
                                kp = 4 * sl + kk
                                q_matmuls(
                                    ps_q, wq_sb,
                                    hch[:, 2 * kk : 2 * kk + 2, 0, :],
                                    hch[:, 2 * kk : 2 * kk + 2, 1, :],
                                    kp,
                                )
                        for m in range(HPC):
                            nc.vector.tensor_scalar_mul(
                                qT[:, m, cs], ps_q[m][:], Q_COPY
                            )


            nc.gpsimd.memset(v_sb[:, :, 128:129], 1.0 / A_AT)
            # k_cmp[t, c'] with t = h*256+u, c' = r*64 + half*32 + j maps to
            #   (half==0 ? KC : VC)[8u + r (+4 for v_cmp), h*32 + j]
            # so the latent operand is a stride-8 slice of kcT/vcT along seq.
            kc_r = kcT.rearrange("p m (u r) -> p m r u", r=8)
            vc_r = vcT.rearrange("p m (u r) -> p m r u", r=8)
            with (
                tc.tile_pool(name="pt", bufs=2) as ptpool,
                tc.tile_pool(name="attn", bufs=2) as apool,
                tc.tile_pool(name="pss", bufs=2, space="PSUM") as pss,
                tc.tile_pool(name="psav", bufs=2, space="PSUM") as psav,
                tc.tile_pool(name="pso", bufs=2, space="PSUM") as pso,
            ):
                pts = {0: [None] * NSQ, 1: [None] * NSQ}

                # ---- up projections; most groups run as PE filler inside
                #      chunk 0's attention loop (o-proj's PSUM banks are idle
                #      there, so their pool is borrowed for the accumulators)
                def k_up(h, pl=None):
                    base = (h % 4) * 32
                    pl = pl or pso
                    ps_up = pl.tile(
                        [128, CHUNK], f32,
                        tag="ps_s" if pl is pss else "ps_o",
                        name="ps_up",
                    )
                    for blk in range(8):
                        r, half = blk // 2, blk % 2
                        src_ = kc_r if half == 0 else vc_r
                        nc.tensor.matmul(
                            ps_up[:, 0:256],
                            lhsT=wkup_sb[base : base + 32, blk * 128 : (blk + 1) * 128],
                            rhs=src_[base : base + 32, h // 4, r, :],
                            start=(blk == 0),
                            stop=(blk == 7),
                            tile_position=(base, 0),
                        )
                    nc.vector.tensor_copy(
                        kT[:, h * 256 : (h + 1) * 256], ps_up[:, 0:256]
                    )

                def v_up(tt, pl=None):
                    h, ub = tt // 2, tt % 2
                    base = (h % 4) * 32
                    pl = pl or pso
                    ps_vt = pl.tile(
                        [128, CHUNK], f32,
                        tag="ps_s" if pl is pss else "ps_o",
                        name="ps_vt",
                    )
                    for blk in range(8):
                        r, half = blk // 2, blk % 2
                        src_ = kc_r if half == 0 else vc_r
                        nc.tensor.matmul(
                            ps_vt[:, 0:128],
                            lhsT=src_[
                                base : base + 32, h // 4, 4 + r,
                                ub * 128 : (ub + 1) * 128,
                            ],
                            rhs=wvup_sb[base : base + 32, blk * 128 : (blk + 1) * 128],
                            start=(blk == 0),
                            stop=(blk == 7),
                            tile_position=(base, 0),
                        )
                    nc.vector.tensor_copy(v_sb[:, tt, 0:128], ps_vt[:, 0:128])

                # prefix: exactly what chunk 0's attention + the interleaved
                # chunk-1 h0 scores/AV need up front
                k_up(0, pss)
                k_up(1, pso)
                for tt in range(4):
                    v_up(tt, pss if tt % 2 else pso)
                filler = (
                    [lambda h=h: k_up(h) for h in range(2, 4)]
                    + [lambda tt=tt: v_up(tt) for tt in range(4, 8)]
                    + [lambda h=h: k_up(h) for h in range(4, 8)]
                    + [lambda tt=tt: v_up(tt) for tt in range(8, NSQ)]
                )
                filler.reverse()  # consumed via .pop()

                def emit_score(c, h, j):
                    s = h % 2
                    q0 = max(0, 128 * (j - 4 * c))
                    ps_s = pss.tile([128, CHUNK], f32, tag="ps_s", name="ps_s")
                    nc.tensor.matmul(
                        ps_s[:, q0:CHUNK],
                        lhsT=kT[:, j * 128 : (j + 1) * 128],
                        rhs=qT[:, h, c * CHUNK + q0 : (c + 1) * CHUNK],
                        start=True,
                        stop=True,
                    )
                    pt = ptpool.tile(
                        [128, CHUNK], bf16, tag=f"pt{s}_{j}", name=f"pt{s}_{j}"
                    )
                    nc.scalar.activation(pt[:, q0:CHUNK], ps_s[:, q0:CHUNK], EXP)
                    if j >= 4 * c:
                        # causal mask: 0/1 multiply on the diagonal 128-block
                        # (post-exp, so the exp never waits on the DVE queue)
                        nc.vector.tensor_mul(
                            pt[:, q0 : q0 + 128], pt[:, q0 : q0 + 128], mask_sb[:]
                        )
                    pts[s][j] = pt

                def attn_qt(c, h, qt):
                    s = h % 2
                    i = 4 * c + qt
                    qs = slice(qt * 128, (qt + 1) * 128)
                    # AV in [q, d|sum]: col 128 accumulates den/A_AT via
                    # v_sb's 1/A_AT column; the per-partition scale on the
                    # PSUM->SBUF copy is then A_AT/den, so at_qd = A_AT*at
                    ps_av = psav.tile([128, 129], f32, tag="ps_av", name="ps_av")
                    for j in range(i + 1):
                        nc.tensor.matmul(
                            ps_av[:],
                            lhsT=pts[s][j][:, qs],
                            rhs=v_sb[:, j, 0:129],
                            start=(j == 0),
                            stop=(j == i),
                        )
                    rec = apool.tile([128, 1], f32, tag="rec", name="rec", bufs=4)
                    nc.vector.reciprocal(rec[:], ps_av[:, 128:129])
                    at_qd = apool.tile(
                        [128, 128], bf16, tag="at_qd", name="at_qd", bufs=6
                    )
                    nc.vector.tensor_scalar_mul(at_qd[:], ps_av[:, 0:128], rec[:])
                    return at_qd

                def emit_transpose(at8, atr8, h, qt, at_qd):
                    qs = slice(qt * 128, (qt + 1) * 128)
                    ps_tr = psav.tile(
                        [128, 128], bf16, tag="ps_tr", name="ps_tr", bufs=2
                    )
                    nc.tensor.transpose(ps_tr[:], at_qd[:], id_sb[:])
                    nc.scalar.copy(at8[:, h, qs], ps_tr[:])
                    nc.vector.tensor_sub(atr8[:, h, qs], ps_tr[:], at8[:, h, qs])

                def o_pair(at8, atr8, i, pn, split=False):
                    out_sb = opool.tile(
                        [128, 2 * CHUNK], bf16, tag="out_sb", name="out_sb"
                    )
                    qs = slice((i % 4) * 128, (i % 4 + 1) * 128)
                    for half in range(2):
                        n = 2 * pn + half
                        ns = slice(n * CHUNK, (n + 1) * CHUNK)
                        ps_o = pso.tile([128, CHUNK], f32, tag="ps_o", name="ps_o")
                        for hh in (0, 2):
                            hs_ = slice(hh, hh + 2)
                            nc.tensor.matmul(
                                ps_o[:], lhsT=at8[:, hs_, qs],
                                rhs=wo_sb[:, hs_, ns], perf_mode=DR,
                                start=(hh == 0), stop=False,
                            )
                            nc.tensor.matmul(
                                ps_o[:], lhsT=atr8[:, hs_, qs],
                                rhs=wo_sb[:, hs_, ns], perf_mode=DR,
                                start=False, stop=False,
                            )
                            nc.tensor.matmul(
                                ps_o[:], lhsT=at8[:, hs_, qs],
                                rhs=rwo_sb[:, hs_, ns], perf_mode=DR,
                                start=False, stop=(hh == 2),
                            )
                        if split and half == 1:
                            nc.vector.tensor_scalar_mul(
                                out_sb[:, CHUNK : CHUNK + 256], ps_o[:, 0:256],
                                O_COPY,
                            )
                            nc.scalar.mul(
                                out_sb[:, CHUNK + 256 : 2 * CHUNK], ps_o[:, 256:512],
                                O_COPY,
                            )
                        else:
                            nc.vector.tensor_scalar_mul(
                                out_sb[:, half * CHUNK : (half + 1) * CHUNK],
                                ps_o[:], O_COPY,
                            )
                        if split and half == 0:
                            nc.sync.dma_start(
                                out=outp[
                                    i * 128 : (i + 1) * 128,
                                    n * CHUNK : (n + 1) * CHUNK,
                                ],
                                in_=out_sb[:, 0:CHUNK],
                            )
                    if not split:
                        (nc.sync, nc.gpsimd, nc.scalar)[(i + pn) % 3].dma_start(
                            out=outp[
                                i * 128 : (i + 1) * 128,
                                2 * pn * CHUNK : 2 * (pn + 1) * CHUNK,
                            ],
                            in_=out_sb[:],
                        )
                    else:
                        nc.scalar.dma_start(
                            out=outp[
                                i * 128 : (i + 1) * 128,
                                (2 * pn + 1) * CHUNK : (2 * pn + 1) * CHUNK + 256,
                            ],
                            in_=out_sb[:, CHUNK : CHUNK + 256],
                        )
                        nc.sync.dma_start(
                            out=outp[
                                i * 128 : (i + 1) * 128,
                                (2 * pn + 1) * CHUNK + 256 : 2 * (pn + 1) * CHUNK,
                            ],
                            in_=out_sb[:, CHUNK + 256 : 2 * CHUNK],
                        )

                prev_at = None  # (at8, atr8) tiles of the previous chunk
                for c in range(NCHUNK):
                    nj = 4 * c + 4
                    at8 = apool.tile([128, HPC, CHUNK], fp8, tag="at8", name="at8")
                    atr8 = apool.tile(
                        [128, HPC, CHUNK], fp8, tag="atr8", name="atr8"
                    )
                    if c == 0:
                        for j in range(nj):
                            emit_score(0, 0, j)
                    slot = 0
                    for h in range(HPC):
                        if h + 1 < HPC:
                            njs = list(range(nj))
                            nxt = (c, h + 1)
                        elif c + 1 < NCHUNK:
                            njs = list(range(4 * (c + 1) + 4))
                            nxt = (c + 1, 0)
                        else:
                            njs, nxt = [], None
                        pending = None
                        for qt in range(4):
                            at_qd = attn_qt(c, h, qt)
                            if prev_at is not None:
                                o_pair(*prev_at, 4 * (c - 1) + slot // 4, slot % 4)
                            lo = (qt * len(njs)) // 4
                            hi = ((qt + 1) * len(njs)) // 4
                            for j in njs[lo:hi]:
                                emit_score(nxt[0], nxt[1], j)
                            if prev_at is None and filler:
                                filler.pop()()
                                if len(filler) > 15 - slot:
                                    filler.pop()()
                            slot += 1
                            if pending is not None:
                                emit_transpose(at8, atr8, h, pending[0], pending[1])
                            pending = (qt, at_qd)
                        emit_transpose(at8, atr8, h, pending[0], pending[1])
                    prev_at = (at8, atr8)
                # last chunk's o-projection runs solo
                for qo in range(4):
                    for pn in range(4):
                        o_pair(
                            *prev_at,
                            4 * (NCHUNK - 1) + qo,
                            pn,
                            split=(qo == 3 and pn == 3),
                        )
    nc.compile()
    return nc


def _f8(x):
    return np.asarray(np.clip(x, -440.0, 440.0), ml_dtypes.float8_e4m3fn)


def _split8(x):
    """fp8 value + same-scale fp8 residual of an f32 array."""
    x8 = _f8(x)
    r8 = _f8(x - x8.astype(np.float32))
    return x8, r8


def _prep_inputs(hidden_states, Wq, Wk_down, Wv_down, Wk_up, Wv_up, Wo):
    bf = ml_dtypes.bfloat16
    hs = np.asarray(hidden_states, dtype=np.float32).reshape(S, HID)
    ht = np.ascontiguousarray(hs.T)  # [HID, S] f32
    ht8, htr8 = _split8(ht * A_HS)
    scale = np.float32(1.0) / np.sqrt(np.float32(D))
    Wq = np.asarray(Wq, dtype=np.float32)
    Wo = np.asarray(Wo, dtype=np.float32)

    def dwn(w):  # [HK*L, HID] -> [128, NKT, HK*L] fp8 pair at A_WD
        wt = np.ascontiguousarray(np.asarray(w, np.float32).T * A_WD)  # [HID, 256]
        wr = np.ascontiguousarray(
            wt.reshape(NKT, 128, HK * L).transpose(1, 0, 2)
        )
        return _split8(wr)

    wkd8, rwkd8 = dwn(Wk_down)
    wvd8, rwvd8 = dwn(Wv_down)

    # transposed causal 0/1 mask: rows t, cols q; allowed where q >= t
    mask = np.where(
        np.arange(128)[None, :] >= np.arange(128)[:, None], 1.0, 0.0
    ).astype(bf)
    identity = np.eye(128, dtype=bf)

    def up_blocks(w):  # w: (128, 256) rows of Wk_up/Wv_up for this core
        arr = np.zeros((128, 8 * 128), np.float32)
        for r in range(4):
            for half in range(2):
                blk = r * 2 + half
                bT = w[:, r * 64 + half * 32 : r * 64 + half * 32 + 32].T
                for b in range(4):
                    arr[b * 32 : (b + 1) * 32, blk * 128 : (blk + 1) * 128] = bT
        return arr.astype(bf)

    in_maps = []
    for c in range(NCORE):
        hm = np.ascontiguousarray(
            (ht[:, c * SSH : (c + 1) * SSH] * A_HS)
            .reshape(NKT, 128, SSH)
            .transpose(1, 0, 2)
        )
        hm8, hmr8 = _split8(hm)
        wqt = np.ascontiguousarray(
            ((Wq[c * 512 : (c + 1) * 512, :] * (scale * A_WQ)).T)
            .reshape(NKT, 128, HPC * D)
            .transpose(1, 0, 2)
        )
        wq8_, rwq8_ = _split8(wqt)
        wkup = up_blocks(np.asarray(Wk_up[c * 128 : (c + 1) * 128, :], np.float32))
        wvup = up_blocks(np.asarray(Wv_up[c * 128 : (c + 1) * 128, :], np.float32))
        wot = np.ascontiguousarray(
            (Wo[:, c * 512 : (c + 1) * 512].T * A_WO)
            .reshape(HPC, 128, HID)
            .transpose(1, 0, 2)
        )
        wo8_, rwo8_ = _split8(wot)
        in_maps.append(
            dict(
                ht8=ht8,
                htr8=htr8,
                hm8=hm8,
                hmr8=hmr8,
                wq8=wq8_,
                rwq8=rwq8_,
                wkd8=wkd8,
                rwkd8=rwkd8,
                wvd8=wvd8,
                rwvd8=rwvd8,
                wkup=wkup,
                wvup=wvup,
                wo8=wo8_,
                rwo8=rwo8_,
                maskt=mask,
                ident=identity,
            )
        )
    return in_maps


def run(trace=False, **inputs):
    from concourse.bass_utils import run_bass_kernel_spmd

    global _BUILT
    if _BUILT is None:
        _BUILT = _build()
    in_maps = _prep_inputs(**inputs)
    res = run_bass_kernel_spmd(
        _BUILT, in_maps, core_ids=list(range(NCORE)), trace=trace
    )
    acc = np.array(res.results[0]["out"], dtype=np.float32, copy=True)
    for r in res.results[1:]:
        acc += np.asarray(r["out"], dtype=np.float32)
    return acc.reshape(B, S, HID), res


def kernel(**inputs):
    out, _ = run(trace=False, **inputs)
    return out
